# revision 21
# baseline (speedup 1.0000x reference)
"""Distributed Trainium2 kernel for AdaptiveConv GNN message passing.

Algorithm (per reference): K=3 iterations of
    agg = A_norm @ x            (SpMM over 1.6M edges + self loops)
    z   = agg - feat
    x   = feat + relu(1 - gl/||z||_row) * z
with A_norm the symmetrically-normalized weighted adjacency (self loops
folded into the epilogue analytically).

Mapping to 8 NeuronCores:
  - dst-node 1D partition: core k owns N/8 destination rows (slot-permuted
    for load balance); edges partitioned by dst core.
  - per-iteration AllGather of the bf16 x-table (256B rows).
  - random src-row gather via SWDGE dma_gather (1024 rows / instruction,
    int16 indices => <=32767-row source chunks), edge-on-partition layout.
  - segment-sum via TensorE matmuls with host-precomputed weighted one-hot
    lhsT [128 edges, 128 dst slots] (bf16), PSUM accumulated per dst group,
    SBUF-accumulated across the chunk passes.
  - prox epilogue (row L2 shrinkage) batched on DVE/ACT in f32.
"""
import numpy as np
import ml_dtypes

from concourse import bass, mybir
import concourse.bacc as bacc
from concourse.bass_utils import run_bass_kernel_spmd
from concourse.library_config import mlp

NCORES = 8
D = 50
K_ITERS = 3
LAM = 0.1
GL = (1.0 / (2.0 * (1.0 - LAM))) * LAM  # gamma * lam
EW = 128          # bf16 table row width -> 256B rows
GB_TILES = 8      # tiles per gather slab (1024 index SWDGE limit)
NBUF = 8          # slab buffer rotation depth
NBANKS = 4        # PSUM bank rotation

BF16 = mybir.dt.bfloat16
F32 = mybir.dt.float32
I16 = mybir.dt.int16

last_exec_time_ns = None


# ----------------------------------------------------------------------------
# host-side preprocessing
# ----------------------------------------------------------------------------

def _pack_slots(degc, groups, caps):
    """Assign local dst ids to (group, slot) packing per-chunk in-degree
    vectors degc [n, nchunk] under per-(group, chunk) caps [groups, nchunk].
    Greedy by total degree; score = cap overflow, then max fill fraction.
    Returns pos[local_id] = group*128 + slot."""
    order = np.argsort(-degc.sum(1), kind="stable")
    loads = np.zeros_like(caps)
    cnts = np.zeros(groups, np.int64)
    pos = np.empty(len(degc), np.int64)
    for lid in order:
        nl = loads + degc[lid]
        over = np.maximum(0, nl - caps).sum(axis=1)
        frac = (nl / caps).max(axis=1)
        score = over * 1e6 + frac + (cnts >= 128) * 1e9
        g = int(np.argmin(score))
        pos[lid] = g * 128 + cnts[g]
        loads[g] += degc[lid]
        cnts[g] += 1
    return pos


def _preprocess(feat, edge_weight, src, dst):
    n, d = feat.shape
    assert d == D and n % NCORES == 0
    shard = n // NCORES
    groups = (shard + 127) // 128
    spad = groups * 128
    trows = NCORES * spad

    # chunk boundaries = shard pairs: chunk membership of a source node is
    # then independent of the slot permutation, enabling per-chunk-balanced
    # slot packing against a shared cap template.
    bounds = [0, 2 * spad, 4 * spad, 6 * spad, trows]
    assert 2 * spad <= 32767
    nchunk = len(bounds) - 1
    bounds_arr = np.asarray(bounds)

    # normalization (degrees include self loops with weight 1)
    ew = edge_weight.astype(np.float64)
    out_deg = np.bincount(src, weights=ew, minlength=n) + 1.0
    in_deg = np.bincount(dst, weights=ew, minlength=n) + 1.0
    iso = out_deg ** -0.5
    isi = in_deg ** -0.5
    w = (ew * iso[src] * isi[dst]).astype(np.float32)
    wself = (iso * isi).astype(np.float32)

    # slot permutation per core: pack per-chunk in-degree vectors under a
    # shared cap template so every (chunk, group) needs the same tile count
    # on every core (the SPMD schedule takes the max over cores).
    dcore = dst // shard
    dloc = dst - dcore * shard
    src_chunk = np.searchsorted(bounds_arr, (src // shard) * spad, side="right") - 1
    base_c = np.zeros(nchunk, np.int64)
    extra_c = np.zeros(nchunk, np.int64)
    cnt_kc = np.zeros((NCORES, nchunk), np.int64)
    np.add.at(cnt_kc, (dcore, src_chunk), 1)
    for c in range(nchunk):
        tc = int(-(-(cnt_kc[:, c].max() * 1.05) // 128))
        base_c[c] = max(1, tc // groups)
        extra_c[c] = tc - base_c[c] * groups
    nbig = int(max(0, extra_c.max()))
    caps = np.tile(base_c, (groups, 1)) * 128
    if nbig:
        caps[groups - nbig:, :] += 128
    pos_all = np.empty(n, np.int64)
    for k in range(NCORES):
        m = dcore == k
        degc = np.zeros((shard, nchunk), np.int64)
        np.add.at(degc, (dloc[m], src_chunk[m]), 1)
        pos_all[k * shard:(k + 1) * shard] = _pack_slots(degc, groups, caps)
    # p-major table rows: row = core*spad + slot*groups + group. Within a
    # partition, consecutive groups are consecutive rows, so the x writeback
    # is one contiguous 25KB descriptor per partition.
    row_all = ((np.arange(n) // shard) * spad
               + (pos_all % 128) * groups + pos_all // 128)

    srow = row_all[src]
    chunk_of = np.searchsorted(bounds_arr, srow, side="right") - 1
    gid = pos_all[dst] // 128
    slot = pos_all[dst] % 128

    # static tile schedule: Tmax[c][g] = max over cores (>= 1)
    counts = np.zeros((NCORES, nchunk, groups), np.int64)
    np.add.at(counts, (dcore, chunk_of, gid), 1)
    tmax = np.maximum(1, -(-counts.max(axis=0) // 128))  # [nchunk, groups]

    tile_base = np.zeros((nchunk, groups), np.int64)
    seq = []          # (c, g) in schedule order
    tiles = []        # per tile: (c, g, j, seq_idx, start, stop)
    t = 0
    for c in range(nchunk):
        for g in range(groups):
            tile_base[c, g] = t
            tm = int(tmax[c, g])
            si = len(seq)
            for j in range(tm):
                tiles.append((c, g, j, si, j == 0, j == tm - 1))
            seq.append((c, g))
            t += tm
    t_total = t

    # slabs: runs of <= GB_TILES tiles, not crossing chunk boundaries
    slabs = []
    for c in range(nchunk):
        c0 = int(tile_base[c, 0])
        c1 = int(tile_base[c + 1, 0]) if c + 1 < nchunk else t_total
        tt = c0
        while tt < c1:
            nt = min(GB_TILES, c1 - tt)
            slabs.append((tt, nt, c))
            tt += nt
    # groups fully completed once a slab's matmuls are all consumed:
    # 1 + seq index of the group containing the slab's last tile
    slab_complete = [tiles[t0 + nt - 1][3] + 1 for (t0, nt, _) in slabs]

    # per-core data
    per_core = []
    for k in range(NCORES):
        m = dcore == k
        e_srow = srow[m]
        e_chunk = chunk_of[m]
        e_g = gid[m]
        e_slot = slot[m]
        e_w = w[m]

        ordk = np.lexsort((e_slot, e_g, e_chunk))
        e_srow, e_chunk, e_g, e_slot, e_w = (
            a[ordk] for a in (e_srow, e_chunk, e_g, e_slot, e_w))
        cnt_k = np.zeros((nchunk, groups), np.int64)
        np.add.at(cnt_k, (e_chunk, e_g), 1)
        assert np.all(cnt_k <= tmax * 128)
        starts = np.zeros(nchunk * groups, np.int64)
        starts[1:] = np.cumsum(cnt_k.ravel())[:-1]
        flat_cg = e_chunk * groups + e_g
        q = np.arange(len(e_w)) - starts[flat_cg]
        tile_idx = tile_base[e_chunk, e_g] + q // 128
        p_idx = q % 128

        idx16 = np.zeros((16, t_total * 8), np.int16)
        idx16[p_idx % 16, tile_idx * 8 + p_idx // 16] = (
            e_srow - bounds_arr[e_chunk]).astype(np.int16)
        # trailing-pad trim: mark trailing pad slots of each slab -1 and
        # record the per-slab valid count (fed to num_idxs_reg at runtime)
        has_edge = np.zeros(t_total * 128, bool)
        has_edge[tile_idx * 128 + p_idx] = True
        gcnt = np.zeros(len(slabs), np.uint32)
        for si_, (t0_, nt_, _c) in enumerate(slabs):
            ne = nt_ * 128
            occ = has_edge[t0_ * 128:t0_ * 128 + ne]
            nz = np.nonzero(occ)[0]
            valid = int(nz[-1]) + 1 if len(nz) else 0
            gcnt[si_] = valid
        idx_arr = np.tile(idx16, (8, 1))

        oneh = np.zeros((128, t_total, 128), np.float32)
        oneh[p_idx, tile_idx, e_slot] = e_w
        oneh_arr = np.ascontiguousarray(
            oneh.reshape(128, t_total * 128)).astype(ml_dtypes.bfloat16)

        inv = np.full(spad, -1, np.int64)
        inv[pos_all[k * shard:(k + 1) * shard]] = np.arange(shard)
        valid = inv >= 0
        feat_pad = np.zeros((spad, D), np.float32)
        feat_pad[valid] = feat[k * shard + inv[valid]]
        wself_pad = np.zeros(spad, np.float32)
        wself_pad[valid] = wself[k * shard + inv[valid]]

        feat_ep = np.ascontiguousarray(
            feat_pad.reshape(groups, 128, D).transpose(1, 0, 2).reshape(128, groups * D))
        wself_bb = np.ascontiguousarray(np.repeat(
            wself_pad.reshape(groups, 128).T[:, :, None], D, axis=2
        ).reshape(128, groups * D))

        xinit = np.zeros((spad, EW), ml_dtypes.bfloat16)
        rr = np.arange(spad)
        xinit[rr, :D] = feat_pad[(rr % groups) * 128 + rr // groups].astype(
            ml_dtypes.bfloat16)

        per_core.append(dict(
            idx=idx_arr, oneh=oneh_arr, feat_ep=feat_ep,
            wselfb=wself_bb.astype(np.float32), xinit=xinit, inv=inv,
            gcnt=gcnt.reshape(1, -1),
        ))

    sched = dict(
        n=n, shard=shard, groups=groups, spad=spad, trows=trows,
        bounds=bounds, nchunk=nchunk, seq=seq, tiles=tiles,
        t_total=t_total, slabs=slabs, slab_complete=slab_complete,
    )
    return per_core, sched


# ----------------------------------------------------------------------------
# device kernel builder
# ----------------------------------------------------------------------------

def _build(sched):
    groups = sched["groups"]
    spad = sched["spad"]
    trows = sched["trows"]
    bounds = sched["bounds"]
    seq = sched["seq"]
    tiles = sched["tiles"]
    slabs = sched["slabs"]
    slab_complete = sched["slab_complete"]
    t_total = sched["t_total"]
    NS = len(slabs)
    NG = len(seq)
    GD = groups * D

    nc = bacc.Bacc("TRN2", num_devices=NCORES)

    tiny = nc.alloc_sbuf_tensor("const-tiny", [128, 1], F32)
    nc.gpsimd.memset(tiny.ap(), 1e-30)
    nc.const_aps.aps[(F32, 1e-30)] = tiny.ap()
    nc.all_engine_barrier()

    idx_ext = nc.declare_dram_parameter("idx", [128, t_total * 8], I16, isOutput=False)
    oneh_ext = nc.declare_dram_parameter("oneh", [128, t_total * 128], BF16, isOutput=False)
    feat_ext = nc.declare_dram_parameter("feat_ep", [128, GD], F32, isOutput=False)
    wself_ext = nc.declare_dram_parameter("wselfb", [128, GD], F32, isOutput=False)
    xinit_ext = nc.declare_dram_parameter("xinit", [spad, EW], BF16, isOutput=False)
    gcnt_ext = nc.declare_dram_parameter("gcnt", [1, NS], mybir.dt.uint32, isOutput=False)
    out_ext = nc.declare_dram_parameter("out", [spad, D], F32, isOutput=True)

    x_bounce = nc.dram_tensor("x_bounce", [spad, EW], BF16)
    x_table = nc.dram_tensor("x_table", [trows, EW], BF16, addr_space="Shared")

    from contextlib import ExitStack
    with ExitStack() as ctx:
        block = ctx.enter_context(nc.Block())
        sem = lambda nm: ctx.enter_context(nc.semaphore(nm))
        sbuf = lambda nm, shp, dt: ctx.enter_context(nc.sbuf_tensor(nm, shp, dt))
        s_init, s_pe, s_dve = sem("s_init"), sem("s_pe"), sem("s_dve")
        s_idx = [sem(f"s_idx{i}") for i in range(NBUF)]
        s_oh = [sem(f"s_oh{i}") for i in range(NBUF)]
        s_gath = [sem(f"s_gath{i}") for i in range(NBUF)]
        s_d2a, s_a2d, s_a2s, s_wb, s_cc, s_vch = (
            sem("s_d2a"), sem("s_a2d"), sem("s_a2s"), sem("s_wb"), sem("s_cc"),
            sem("s_vch"))
        gath_buf = sbuf("gath_buf", [128, NBUF * GB_TILES, EW], BF16)
        oneh_buf = sbuf("oneh_buf", [128, NBUF * GB_TILES * 128], BF16)
        idx_buf = sbuf("idx_buf", [128, NBUF * GB_TILES * 8], I16)
        feat_sb = sbuf("feat_sb", [128, GD], F32)
        wself_sb = sbuf("wself_sb", [128, GD], F32)
        x_sb = sbuf("x_sb", [128, GD], F32)
        agg_sb = sbuf("agg_sb", [128, GD], F32)
        s1_sb = sbuf("s1_sb", [128, GD], F32)
        s2_sb = sbuf("s2_sb", [128, GD], F32)
        xbf_sb = sbuf("xbf_sb", [128, groups * 128], BF16)
        ss_sb = sbuf("ss_sb", [128, groups], F32)
        norm_sb = sbuf("norm_sb", [128, groups], F32)
        rinv_sb = sbuf("rinv_sb", [128, groups], F32)
        scale_sb = sbuf("scale_sb", [128, groups], F32)
        gcnt_sb = sbuf("gcnt_sb", [1, NS], mybir.dt.uint32)
        psum = [ctx.enter_context(nc.psum_tensor(f"psum{i}", [128, 512], F32))
                for i in range(NBANKS)]


        @block.sync
        def _(sync: bass.BassEngine):
            sync.dma_start(out=feat_sb[:, :], in_=feat_ext[:, :]).then_inc(s_init, 16)
            sync.dma_start(out=wself_sb[:, :], in_=wself_ext[:, :]).then_inc(s_init, 16)
            sync.dma_start(out=x_bounce[:, :], in_=xinit_ext[:, :]).then_inc(s_init, 16)
            sync.dma_start(out=gcnt_sb[:, :], in_=gcnt_ext[:, :]).then_inc(s_init, 16)
            for k in range(K_ITERS):
                for s, (t0, nt, c) in enumerate(slabs):
                    gs = k * NS + s
                    b = gs % NBUF
                    if gs >= NBUF:
                        # idx buffer consumed by the gather instruction itself
                        sync.wait_ge(s_gath[b], 16 * (gs // NBUF))
                    sync.dma_start(
                        out=idx_buf[:, b * (GB_TILES * 8):b * (GB_TILES * 8) + nt * 8],
                        in_=idx_ext[:, t0 * 8:(t0 + nt) * 8],
                    ).then_inc(s_idx[b], 16)
                    if gs >= NBUF:
                        # oneh buffer consumed by tensor engine matmuls
                        prev = gs - NBUF
                        sync.wait_ge(s_pe, (prev // NS) * NG + slab_complete[prev % NS])
                    sync.dma_start(
                        out=oneh_buf[:, (b * GB_TILES) * 128:(b * GB_TILES + nt) * 128],
                        in_=oneh_ext[:, t0 * 128:(t0 + nt) * 128],
                    ).then_inc(s_oh[b], 16)
                if k < K_ITERS - 1:
                    sync.wait_ge(s_a2s, k + 1)
                    sync.dma_start(
                        out=x_bounce.ap().rearrange("(p g) c -> p (g c)", g=groups),
                        in_=xbf_sb[:, :],
                    ).then_inc(s_wb, 16)
            sync.wait_ge(s_d2a, K_ITERS * 3)
            sync.dma_start(
                out=out_ext.ap().rearrange("(p g) c -> p (g c)", g=groups),
                in_=x_sb[:, :],
            ).then_inc(s_wb, 16)
            sync.wait_ge(s_wb, 16 * K_ITERS)

        @block.gpsimd
        def _(gpsimd: bass.BassGpSimd):
            gpsimd.load_library(mlp)
            gpsimd.wait_ge(s_init, 66)
            nreg = gpsimd.alloc_register("gcnt_reg")
            for k in range(K_ITERS):
                if k > 0:
                    gpsimd.wait_ge(s_wb, 16 * k)
                gpsimd.collective_compute(
                    "AllGather",
                    mybir.AluOpType.bypass,
                    replica_groups=[list(range(NCORES))],
                    ins=[x_bounce.ap().opt()],
                    outs=[x_table.ap().opt()],
                ).then_inc(s_cc)
                gpsimd.wait_ge(s_cc, k + 1)
                for s, (t0, nt, c) in enumerate(slabs):
                    gs = k * NS + s
                    b = gs % NBUF
                    gpsimd.wait_ge(s_idx[b], 16 * (gs // NBUF + 1))
                    if gs >= NBUF:
                        prev = gs - NBUF
                        gpsimd.wait_ge(s_pe, (prev // NS) * NG + slab_complete[prev % NS])
                    gpsimd.dma_gather(
                        out_ap=gath_buf[:, b * GB_TILES:b * GB_TILES + nt, :],
                        in_ap=x_table[bounds[c]:bounds[c + 1], :],
                        idxs_ap=idx_buf[:, b * (GB_TILES * 8):b * (GB_TILES * 8) + nt * 8],
                        num_idxs=nt * 128,
                        num_idxs_reg=nt * 128,
                        elem_size=EW,
                    ).then_inc(s_gath[b], 16)

        @block.tensor
        def _(tensor: bass.BassEngine):
            for k in range(K_ITERS):
                for s, (t0, nt, c) in enumerate(slabs):
                    gs = k * NS + s
                    b = gs % NBUF
                    tensor.wait_ge(s_gath[b], 16 * (gs // NBUF + 1))
                    tensor.wait_ge(s_oh[b], 16 * (gs // NBUF + 1))
                    for j in range(nt):
                        t = t0 + j
                        _, g, _, si, is_start, is_stop = tiles[t]
                        gsi = k * NG + si
                        bank = si % NBANKS
                        if is_start and gsi >= NBANKS:
                            tensor.wait_ge(s_dve, gsi - NBANKS + 1)
                        mm = tensor.matmul(
                            out=psum[bank][:, 0:D],
                            lhsT=oneh_buf[:, (b * GB_TILES + j) * 128:(b * GB_TILES + j + 1) * 128],
                            rhs=gath_buf[:, b * GB_TILES + j, 0:D],
                            start=is_start, stop=is_stop,
                            tile_position=(0, 0),
                        )
                        if is_stop:
                            mm.then_inc(s_pe, 1)

        @block.vector
        def _(vector: bass.BassEngine):
            vc = 0
            vector.memset(gath_buf[:, :, :], 0.0).then_inc(s_init, 1)
            vector.memset(xbf_sb[:, :], 0.0).then_inc(s_init, 1)
            vector.wait_ge(s_init, 66)
            for k in range(K_ITERS):
                for i, (c, g) in enumerate(seq):
                    bank = i % NBANKS
                    vector.wait_ge(s_pe, k * NG + i + 1)
                    if c > 0:
                        # prior drain of same group must have landed
                        vector.wait_ge(s_dve, k * NG + i - groups + 1)
                    gsl = slice(g * D, (g + 1) * D)
                    if c == 0:
                        op = vector.tensor_copy(out=agg_sb[:, gsl], in_=psum[bank][:, 0:D])
                    else:
                        op = vector.tensor_tensor(
                            out=agg_sb[:, gsl], in0=agg_sb[:, gsl],
                            in1=psum[bank][:, 0:D], op=mybir.AluOpType.add)
                    op.then_inc(s_dve, 1)
                # epilogue (batched, f32); wait for own drain writes to land
                vector.wait_ge(s_dve, (k + 1) * NG)
                xsrc = feat_sb if k == 0 else x_sb
                vector.tensor_tensor(out=s1_sb[:, :], in0=xsrc[:, :], in1=wself_sb[:, :],
                                     op=mybir.AluOpType.mult).then_inc(s_vch, 1)
                vc += 1
                vector.wait_ge(s_vch, vc)
                vector.tensor_tensor(out=s2_sb[:, :], in0=agg_sb[:, :], in1=s1_sb[:, :],
                                     op=mybir.AluOpType.add).then_inc(s_vch, 1)
                vc += 1
                vector.wait_ge(s_vch, vc)
                vector.tensor_tensor(out=s1_sb[:, :], in0=s2_sb[:, :], in1=feat_sb[:, :],
                                     op=mybir.AluOpType.subtract).then_inc(s_vch, 1)  # z
                vc += 1
                vector.wait_ge(s_vch, vc)
                vector.tensor_tensor(out=s2_sb[:, :], in0=s1_sb[:, :], in1=s1_sb[:, :],
                                     op=mybir.AluOpType.mult).then_inc(s_vch, 1)  # z^2
                vc += 1
                vector.wait_ge(s_vch, vc)
                vector.tensor_reduce(
                    out=ss_sb[:, :],
                    in_=s2_sb.ap().rearrange("p (g c) -> p g c", c=D),
                    axis=mybir.AxisListType.X, op=mybir.AluOpType.add,
                ).then_inc(s_d2a, 1)
                vector.wait_ge(s_a2d, k * 2 + 1)
                vector.reciprocal(out=rinv_sb[:, :], in_=norm_sb[:, :]).then_inc(s_d2a, 1)
                vector.wait_ge(s_a2d, k * 2 + 2)
                for g in range(groups):
                    gsl = slice(g * D, (g + 1) * D)
                    op = vector.tensor_scalar(
                        out=s2_sb[:, gsl], in0=s1_sb[:, gsl],
                        scalar1=scale_sb[:, g:g + 1], scalar2=None,
                        op0=mybir.AluOpType.mult)
                op.then_inc(s_vch, 1)
                vc += 1
                vector.wait_ge(s_vch, vc)
                vector.tensor_tensor(out=x_sb[:, :], in0=s2_sb[:, :], in1=feat_sb[:, :],
                                     op=mybir.AluOpType.add).then_inc(s_d2a, 1)

        @block.scalar
        def _(scalar: bass.BassEngine):
            for k in range(K_ITERS):
                scalar.wait_ge(s_d2a, k * 3 + 1)
                scalar.activation(out=norm_sb[:, :], in_=ss_sb[:, :],
                                  func=mybir.ActivationFunctionType.Sqrt,
                                  bias=1e-30).then_inc(s_a2d, 1)
                scalar.wait_ge(s_d2a, k * 3 + 2)
                scalar.activation(out=scale_sb[:, :], in_=rinv_sb[:, :],
                                  func=mybir.ActivationFunctionType.Relu,
                                  bias=1.0, scale=-float(GL)).then_inc(s_a2d, 1)
                if k < K_ITERS - 1:
                    scalar.wait_ge(s_d2a, k * 3 + 3)
                    if k > 0:
                        scalar.wait_ge(s_wb, 16 * k)
                    _x3 = xbf_sb.ap().rearrange("p (g c) -> p g c", c=128)
                    _xo = bass.AP(_x3.tensor, _x3.offset,
                                  [list(_x3.ap[0]), list(_x3.ap[1]), [1, D]])
                    scalar.activation(out=_xo,
                                      in_=x_sb.ap().rearrange("p (g c) -> p g c", c=D),
                                      func=mybir.ActivationFunctionType.Copy).then_inc(s_a2s, 1)

    nc.compile()
    return nc


# ----------------------------------------------------------------------------
# public entry point
# ----------------------------------------------------------------------------

def _install_ntff_hook_shim():
    """Provide antenv.axon_hooks (missing in this image) so
    run_bass_kernel_spmd(trace=True) can capture an NTFF profile."""
    import sys, types
    try:
        import antenv.axon_hooks  # noqa: F401
        return
    except ImportError:
        pass
    if "antenv.axon_hooks" in sys.modules:
        return
    try:
        from trn_agent_boot.trn_boot import _ntff_profile_via_ctypes
        hook = _ntff_profile_via_ctypes("/opt/axon/libaxon_pjrt.so")
    except Exception:
        hook = None
    m = types.ModuleType("antenv.axon_hooks")
    m.get_axon_ntff_profile_hook = lambda: hook
    m.set_axon_ntff_profile_hook = lambda h: None
    sys.modules["antenv.axon_hooks"] = m


def kernel(feat, edge_weight, src, dst):
    global last_exec_time_ns
    feat = np.asarray(feat, np.float32)
    edge_weight = np.asarray(edge_weight, np.float32)
    src = np.asarray(src, np.int32)
    dst = np.asarray(dst, np.int32)

    per_core, sched = _preprocess(feat, edge_weight, src, dst)
    nc = _build(sched)

    in_maps = [
        {k: v for k, v in pc.items() if k != "inv"}
        for pc in per_core
    ]
    import os
    if os.environ.get("KERNEL_SIM"):
        import concourse.bass_interp as bass_interp
        sim = bass_interp.MultiCoreSim(nc, NCORES)
        for i in range(NCORES):
            for name, arr in in_maps[i].items():
                sim.cores[i].tensor(name)[:] = arr
        sim.simulate()
        outs = [np.asarray(sim.cores[i].mem_tensor("out")) for i in range(NCORES)]
    else:
        trace = os.environ.get("KERNEL_TRACE", "0") != "0"
        res = None
        if trace:
            try:
                _install_ntff_hook_shim()
                res = run_bass_kernel_spmd(nc, in_maps, core_ids=list(range(NCORES)),
                                           trace=True)
                last_exec_time_ns = res.exec_time_ns
            except Exception:
                res = None
        if res is None:
            res = run_bass_kernel_spmd(nc, in_maps, core_ids=list(range(NCORES)))
        outs = [res.results[k]["out"] for k in range(NCORES)]

    shard = sched["shard"]
    groups = sched["groups"]
    spad = sched["spad"]
    pos_ar = np.arange(spad)
    row_of_pos = (pos_ar % 128) * groups + pos_ar // 128
    out = np.empty((sched["n"], D), np.float32)
    for k in range(NCORES):
        o = outs[k][row_of_pos]  # reorder p-major rows back to slot order
        inv = per_core[k]["inv"]
        valid = inv >= 0
        out[k * shard + inv[valid]] = o[valid]
    return out


# revision 22
# speedup vs baseline: 1.1036x; 1.1036x over previous
"""Distributed Trainium2 kernel for AdaptiveConv GNN message passing.

Algorithm (per reference): K=3 iterations of
    agg = A_norm @ x            (SpMM over 1.6M edges + self loops)
    z   = agg - feat
    x   = feat + relu(1 - gl/||z||_row) * z
with A_norm the symmetrically-normalized weighted adjacency (self loops
folded into the epilogue analytically).

Mapping to 8 NeuronCores:
  - dst-node 1D partition: core k owns N/8 destination rows (slot-permuted
    for load balance); edges partitioned by dst core.
  - per-iteration AllGather of the bf16 x-table (256B rows).
  - random src-row gather via SWDGE dma_gather (1024 rows / instruction,
    int16 indices => <=32767-row source chunks), edge-on-partition layout.
  - segment-sum via TensorE matmuls with host-precomputed weighted one-hot
    lhsT [128 edges, 128 dst slots] (bf16), PSUM accumulated per dst group,
    SBUF-accumulated across the chunk passes.
  - prox epilogue (row L2 shrinkage) batched on DVE/ACT in f32.
"""
import numpy as np
import ml_dtypes

from concourse import bass, mybir
import concourse.bacc as bacc
from concourse.bass_utils import run_bass_kernel_spmd
from concourse.library_config import mlp

NCORES = 8
D = 50
K_ITERS = 3
LAM = 0.1
GL = (1.0 / (2.0 * (1.0 - LAM))) * LAM  # gamma * lam
EW = 128          # bf16 table row width -> 256B rows
GB_TILES = 8      # tiles per gather slab (1024 index SWDGE limit)
NBUF = 8          # slab buffer rotation depth
NBANKS = 4        # PSUM bank rotation

BF16 = mybir.dt.bfloat16
F32 = mybir.dt.float32
I16 = mybir.dt.int16

last_exec_time_ns = None


# ----------------------------------------------------------------------------
# host-side preprocessing
# ----------------------------------------------------------------------------

def _pack_slots(degc, groups, caps):
    """Assign local dst ids to (group, slot) packing per-chunk in-degree
    vectors degc [n, nchunk] under per-(group, chunk) caps [groups, nchunk].
    Greedy by total degree; score = cap overflow, then max fill fraction.
    Returns pos[local_id] = group*128 + slot."""
    order = np.argsort(-degc.sum(1), kind="stable")
    loads = np.zeros_like(caps)
    cnts = np.zeros(groups, np.int64)
    pos = np.empty(len(degc), np.int64)
    for lid in order:
        nl = loads + degc[lid]
        over = np.maximum(0, nl - caps).sum(axis=1)
        frac = (nl / caps).max(axis=1)
        score = over * 1e6 + frac + (cnts >= 128) * 1e9
        g = int(np.argmin(score))
        pos[lid] = g * 128 + cnts[g]
        loads[g] += degc[lid]
        cnts[g] += 1
    return pos


def _preprocess(feat, edge_weight, src, dst):
    n, d = feat.shape
    assert d == D and n % NCORES == 0
    shard = n // NCORES
    groups = (shard + 127) // 128
    spad = groups * 128
    trows = NCORES * spad

    # chunk boundaries = shard pairs: chunk membership of a source node is
    # then independent of the slot permutation, enabling per-chunk-balanced
    # slot packing against a shared cap template.
    bounds = [0, 2 * spad, 4 * spad, 6 * spad, trows]
    assert 2 * spad <= 32767
    nchunk = len(bounds) - 1
    bounds_arr = np.asarray(bounds)

    # normalization (degrees include self loops with weight 1)
    ew = edge_weight.astype(np.float64)
    out_deg = np.bincount(src, weights=ew, minlength=n) + 1.0
    in_deg = np.bincount(dst, weights=ew, minlength=n) + 1.0
    iso = out_deg ** -0.5
    isi = in_deg ** -0.5
    w = (ew * iso[src] * isi[dst]).astype(np.float32)
    wself = (iso * isi).astype(np.float32)

    # slot permutation per core: pack per-chunk in-degree vectors under a
    # shared cap template so every (chunk, group) needs the same tile count
    # on every core (the SPMD schedule takes the max over cores).
    dcore = dst // shard
    dloc = dst - dcore * shard
    src_chunk = np.searchsorted(bounds_arr, (src // shard) * spad, side="right") - 1
    base_c = np.zeros(nchunk, np.int64)
    extra_c = np.zeros(nchunk, np.int64)
    cnt_kc = np.zeros((NCORES, nchunk), np.int64)
    np.add.at(cnt_kc, (dcore, src_chunk), 1)
    for c in range(nchunk):
        tc = int(-(-(cnt_kc[:, c].max() * 1.05) // 128))
        base_c[c] = max(1, tc // groups)
        extra_c[c] = tc - base_c[c] * groups
    nbig = int(max(0, extra_c.max()))
    caps = np.tile(base_c, (groups, 1)) * 128
    if nbig:
        caps[groups - nbig:, :] += 128
    pos_all = np.empty(n, np.int64)
    for k in range(NCORES):
        m = dcore == k
        degc = np.zeros((shard, nchunk), np.int64)
        np.add.at(degc, (dloc[m], src_chunk[m]), 1)
        pos_all[k * shard:(k + 1) * shard] = _pack_slots(degc, groups, caps)
    row_all = (np.arange(n) // shard) * spad + pos_all  # node -> table row

    srow = row_all[src]
    chunk_of = np.searchsorted(bounds_arr, srow, side="right") - 1
    gid = pos_all[dst] // 128
    slot = pos_all[dst] % 128

    # static tile schedule: Tmax[c][g] = max over cores (>= 1)
    counts = np.zeros((NCORES, nchunk, groups), np.int64)
    np.add.at(counts, (dcore, chunk_of, gid), 1)
    tmax = np.maximum(1, -(-counts.max(axis=0) // 128))  # [nchunk, groups]

    tile_base = np.zeros((nchunk, groups), np.int64)
    seq = []          # (c, g) in schedule order
    tiles = []        # per tile: (c, g, j, seq_idx, start, stop)
    t = 0
    for c in range(nchunk):
        for g in range(groups):
            tile_base[c, g] = t
            tm = int(tmax[c, g])
            si = len(seq)
            for j in range(tm):
                tiles.append((c, g, j, si, j == 0, j == tm - 1))
            seq.append((c, g))
            t += tm
    t_total = t

    # slabs: runs of <= GB_TILES tiles, not crossing chunk boundaries
    slabs = []
    for c in range(nchunk):
        c0 = int(tile_base[c, 0])
        c1 = int(tile_base[c + 1, 0]) if c + 1 < nchunk else t_total
        tt = c0
        while tt < c1:
            nt = min(GB_TILES, c1 - tt)
            slabs.append((tt, nt, c))
            tt += nt
    # groups fully completed once a slab's matmuls are all consumed:
    # 1 + seq index of the group containing the slab's last tile
    slab_complete = [tiles[t0 + nt - 1][3] + 1 for (t0, nt, _) in slabs]

    # per-core data
    per_core = []
    for k in range(NCORES):
        m = dcore == k
        e_srow = srow[m]
        e_chunk = chunk_of[m]
        e_g = gid[m]
        e_slot = slot[m]
        e_w = w[m]

        ordk = np.lexsort((e_slot, e_g, e_chunk))
        e_srow, e_chunk, e_g, e_slot, e_w = (
            a[ordk] for a in (e_srow, e_chunk, e_g, e_slot, e_w))
        cnt_k = np.zeros((nchunk, groups), np.int64)
        np.add.at(cnt_k, (e_chunk, e_g), 1)
        assert np.all(cnt_k <= tmax * 128)
        starts = np.zeros(nchunk * groups, np.int64)
        starts[1:] = np.cumsum(cnt_k.ravel())[:-1]
        flat_cg = e_chunk * groups + e_g
        q = np.arange(len(e_w)) - starts[flat_cg]
        tile_idx = tile_base[e_chunk, e_g] + q // 128
        p_idx = q % 128

        idx16 = np.zeros((16, t_total * 8), np.int16)
        idx16[p_idx % 16, tile_idx * 8 + p_idx // 16] = (
            e_srow - bounds_arr[e_chunk]).astype(np.int16)
        # trailing-pad trim: mark trailing pad slots of each slab -1 and
        # record the per-slab valid count (fed to num_idxs_reg at runtime)
        has_edge = np.zeros(t_total * 128, bool)
        has_edge[tile_idx * 128 + p_idx] = True
        gcnt = np.zeros(len(slabs), np.uint32)
        for si_, (t0_, nt_, _c) in enumerate(slabs):
            ne = nt_ * 128
            occ = has_edge[t0_ * 128:t0_ * 128 + ne]
            nz = np.nonzero(occ)[0]
            valid = int(nz[-1]) + 1 if len(nz) else 0
            gcnt[si_] = valid
        idx_arr = np.tile(idx16, (8, 1))

        oneh = np.zeros((128, t_total, 128), np.float32)
        oneh[p_idx, tile_idx, e_slot] = e_w
        oneh_arr = np.ascontiguousarray(
            oneh.reshape(128, t_total * 128)).astype(ml_dtypes.bfloat16)

        inv = np.full(spad, -1, np.int64)
        inv[pos_all[k * shard:(k + 1) * shard]] = np.arange(shard)
        valid = inv >= 0
        feat_pad = np.zeros((spad, D), np.float32)
        feat_pad[valid] = feat[k * shard + inv[valid]]
        wself_pad = np.zeros(spad, np.float32)
        wself_pad[valid] = wself[k * shard + inv[valid]]

        feat_ep = np.ascontiguousarray(
            feat_pad.reshape(groups, 128, D).transpose(1, 0, 2).reshape(128, groups * D))
        wself_bb = np.ascontiguousarray(np.repeat(
            wself_pad.reshape(groups, 128).T[:, :, None], D, axis=2
        ).reshape(128, groups * D))

        xinit = np.zeros((spad, EW), ml_dtypes.bfloat16)
        xinit[:, :D] = feat_pad.astype(ml_dtypes.bfloat16)

        per_core.append(dict(
            idx=idx_arr, oneh=oneh_arr, feat_ep=feat_ep,
            wselfb=wself_bb.astype(np.float32), xinit=xinit, inv=inv,
            gcnt=gcnt.reshape(1, -1),
        ))

    sched = dict(
        n=n, shard=shard, groups=groups, spad=spad, trows=trows,
        bounds=bounds, nchunk=nchunk, seq=seq, tiles=tiles,
        t_total=t_total, slabs=slabs, slab_complete=slab_complete,
    )
    return per_core, sched


# ----------------------------------------------------------------------------
# device kernel builder
# ----------------------------------------------------------------------------

def _build(sched):
    groups = sched["groups"]
    spad = sched["spad"]
    trows = sched["trows"]
    bounds = sched["bounds"]
    seq = sched["seq"]
    tiles = sched["tiles"]
    slabs = sched["slabs"]
    slab_complete = sched["slab_complete"]
    t_total = sched["t_total"]
    NS = len(slabs)
    NG = len(seq)
    GD = groups * D

    nc = bacc.Bacc("TRN2", num_devices=NCORES)

    tiny = nc.alloc_sbuf_tensor("const-tiny", [128, 1], F32)
    nc.gpsimd.memset(tiny.ap(), 1e-30)
    nc.const_aps.aps[(F32, 1e-30)] = tiny.ap()
    nc.all_engine_barrier()

    idx_ext = nc.declare_dram_parameter("idx", [128, t_total * 8], I16, isOutput=False)
    oneh_ext = nc.declare_dram_parameter("oneh", [128, t_total * 128], BF16, isOutput=False)
    feat_ext = nc.declare_dram_parameter("feat_ep", [128, GD], F32, isOutput=False)
    wself_ext = nc.declare_dram_parameter("wselfb", [128, GD], F32, isOutput=False)
    xinit_ext = nc.declare_dram_parameter("xinit", [spad, EW], BF16, isOutput=False)
    gcnt_ext = nc.declare_dram_parameter("gcnt", [1, NS], mybir.dt.uint32, isOutput=False)
    out_ext = nc.declare_dram_parameter("out", [spad, D], F32, isOutput=True)

    x_bounce = nc.dram_tensor("x_bounce", [spad, EW], BF16)
    x_table = nc.dram_tensor("x_table", [trows, EW], BF16, addr_space="Shared")

    from contextlib import ExitStack
    with ExitStack() as ctx:
        block = ctx.enter_context(nc.Block())
        sem = lambda nm: ctx.enter_context(nc.semaphore(nm))
        sbuf = lambda nm, shp, dt: ctx.enter_context(nc.sbuf_tensor(nm, shp, dt))
        s_init, s_pe, s_dve = sem("s_init"), sem("s_pe"), sem("s_dve")
        s_idx = [sem(f"s_idx{i}") for i in range(NBUF)]
        s_oh = [sem(f"s_oh{i}") for i in range(NBUF)]
        s_gath = [sem(f"s_gath{i}") for i in range(NBUF)]
        s_d2a, s_a2d, s_a2s, s_wb, s_cc, s_vch = (
            sem("s_d2a"), sem("s_a2d"), sem("s_a2s"), sem("s_wb"), sem("s_cc"),
            sem("s_vch"))
        gath_buf = sbuf("gath_buf", [128, NBUF * GB_TILES, EW], BF16)
        oneh_buf = sbuf("oneh_buf", [128, NBUF * GB_TILES * 128], BF16)
        idx_buf = sbuf("idx_buf", [128, NBUF * GB_TILES * 8], I16)
        feat_sb = sbuf("feat_sb", [128, GD], F32)
        wself_sb = sbuf("wself_sb", [128, GD], F32)
        x_sb = sbuf("x_sb", [128, GD], F32)
        agg_sb = sbuf("agg_sb", [128, GD], F32)
        s1_sb = sbuf("s1_sb", [128, GD], F32)
        s2_sb = sbuf("s2_sb", [128, GD], F32)
        xbf_sb = sbuf("xbf_sb", [128, GD], BF16)
        ss_sb = sbuf("ss_sb", [128, groups], F32)
        norm_sb = sbuf("norm_sb", [128, groups], F32)
        rinv_sb = sbuf("rinv_sb", [128, groups], F32)
        scale_sb = sbuf("scale_sb", [128, groups], F32)
        gcnt_sb = sbuf("gcnt_sb", [1, NS], mybir.dt.uint32)
        psum = [ctx.enter_context(nc.psum_tensor(f"psum{i}", [128, 512], F32))
                for i in range(NBANKS)]


        @block.sync
        def _(sync: bass.BassEngine):
            sync.dma_start(out=feat_sb[:, :], in_=feat_ext[:, :]).then_inc(s_init, 16)
            sync.dma_start(out=wself_sb[:, :], in_=wself_ext[:, :]).then_inc(s_init, 16)
            sync.dma_start(out=x_bounce[:, :], in_=xinit_ext[:, :]).then_inc(s_init, 16)
            sync.dma_start(out=gcnt_sb[:, :], in_=gcnt_ext[:, :]).then_inc(s_init, 16)
            for k in range(K_ITERS):
                for s, (t0, nt, c) in enumerate(slabs):
                    gs = k * NS + s
                    b = gs % NBUF
                    if gs >= NBUF:
                        # idx buffer consumed by the gather instruction itself
                        sync.wait_ge(s_gath[b], 16 * (gs // NBUF))
                    sync.dma_start(
                        out=idx_buf[:, b * (GB_TILES * 8):b * (GB_TILES * 8) + nt * 8],
                        in_=idx_ext[:, t0 * 8:(t0 + nt) * 8],
                    ).then_inc(s_idx[b], 16)
                    if gs >= NBUF:
                        # oneh buffer consumed by tensor engine matmuls
                        prev = gs - NBUF
                        sync.wait_ge(s_pe, (prev // NS) * NG + slab_complete[prev % NS])
                    sync.dma_start(
                        out=oneh_buf[:, (b * GB_TILES) * 128:(b * GB_TILES + nt) * 128],
                        in_=oneh_ext[:, t0 * 128:(t0 + nt) * 128],
                    ).then_inc(s_oh[b], 16)
                if k < K_ITERS - 1:
                    sync.wait_ge(s_a2s, k + 1)
                    sync.dma_start(
                        out=x_bounce.ap()[:, :D].rearrange("(g p) c -> p g c", p=128),
                        in_=xbf_sb.ap().rearrange("p (g c) -> p g c", c=D),
                    ).then_inc(s_wb, 16)
            sync.wait_ge(s_d2a, K_ITERS * 3)
            sync.dma_start(
                out=out_ext.ap().rearrange("(g p) c -> p g c", p=128),
                in_=x_sb.ap().rearrange("p (g c) -> p g c", c=D),
            ).then_inc(s_wb, 16)
            sync.wait_ge(s_wb, 16 * K_ITERS)

        @block.gpsimd
        def _(gpsimd: bass.BassGpSimd):
            gpsimd.load_library(mlp)
            gpsimd.wait_ge(s_init, 65)
            nreg = gpsimd.alloc_register("gcnt_reg")
            for k in range(K_ITERS):
                if k > 0:
                    gpsimd.wait_ge(s_wb, 16 * k)
                gpsimd.collective_compute(
                    "AllGather",
                    mybir.AluOpType.bypass,
                    replica_groups=[list(range(NCORES))],
                    ins=[x_bounce.ap().opt()],
                    outs=[x_table.ap().opt()],
                ).then_inc(s_cc)
                gpsimd.wait_ge(s_cc, k + 1)
                for s, (t0, nt, c) in enumerate(slabs):
                    gs = k * NS + s
                    b = gs % NBUF
                    gpsimd.wait_ge(s_idx[b], 16 * (gs // NBUF + 1))
                    if gs >= NBUF:
                        prev = gs - NBUF
                        gpsimd.wait_ge(s_pe, (prev // NS) * NG + slab_complete[prev % NS])
                    gpsimd.dma_gather(
                        out_ap=gath_buf[:, b * GB_TILES:b * GB_TILES + nt, :],
                        in_ap=x_table[bounds[c]:bounds[c + 1], :],
                        idxs_ap=idx_buf[:, b * (GB_TILES * 8):b * (GB_TILES * 8) + nt * 8],
                        num_idxs=nt * 128,
                        num_idxs_reg=nt * 128,
                        elem_size=EW,
                    ).then_inc(s_gath[b], 16)

        @block.tensor
        def _(tensor: bass.BassEngine):
            for k in range(K_ITERS):
                for s, (t0, nt, c) in enumerate(slabs):
                    gs = k * NS + s
                    b = gs % NBUF
                    tensor.wait_ge(s_gath[b], 16 * (gs // NBUF + 1))
                    tensor.wait_ge(s_oh[b], 16 * (gs // NBUF + 1))
                    for j in range(nt):
                        t = t0 + j
                        _, g, _, si, is_start, is_stop = tiles[t]
                        gsi = k * NG + si
                        bank = si % NBANKS
                        if is_start and gsi >= NBANKS:
                            tensor.wait_ge(s_dve, gsi - NBANKS + 1)
                        mm = tensor.matmul(
                            out=psum[bank][:, 0:D],
                            lhsT=oneh_buf[:, (b * GB_TILES + j) * 128:(b * GB_TILES + j + 1) * 128],
                            rhs=gath_buf[:, b * GB_TILES + j, 0:D],
                            start=is_start, stop=is_stop,
                            tile_position=(0, 0),
                        )
                        if is_stop:
                            mm.then_inc(s_pe, 1)

        @block.vector
        def _(vector: bass.BassEngine):
            vc = 0
            vector.memset(gath_buf[:, :, :], 0.0).then_inc(s_init, 1)
            vector.wait_ge(s_init, 65)
            for k in range(K_ITERS):
                for i, (c, g) in enumerate(seq):
                    bank = i % NBANKS
                    vector.wait_ge(s_pe, k * NG + i + 1)
                    if c > 0:
                        # prior drain of same group must have landed
                        vector.wait_ge(s_dve, k * NG + i - groups + 1)
                    gsl = slice(g * D, (g + 1) * D)
                    if c == 0:
                        op = vector.tensor_copy(out=agg_sb[:, gsl], in_=psum[bank][:, 0:D])
                    else:
                        op = vector.tensor_tensor(
                            out=agg_sb[:, gsl], in0=agg_sb[:, gsl],
                            in1=psum[bank][:, 0:D], op=mybir.AluOpType.add)
                    op.then_inc(s_dve, 1)
                # epilogue (batched, f32); wait for own drain writes to land
                vector.wait_ge(s_dve, (k + 1) * NG)
                xsrc = feat_sb if k == 0 else x_sb
                vector.tensor_tensor(out=s1_sb[:, :], in0=xsrc[:, :], in1=wself_sb[:, :],
                                     op=mybir.AluOpType.mult).then_inc(s_vch, 1)
                vc += 1
                vector.wait_ge(s_vch, vc)
                vector.tensor_tensor(out=s2_sb[:, :], in0=agg_sb[:, :], in1=s1_sb[:, :],
                                     op=mybir.AluOpType.add).then_inc(s_vch, 1)
                vc += 1
                vector.wait_ge(s_vch, vc)
                vector.tensor_tensor(out=s1_sb[:, :], in0=s2_sb[:, :], in1=feat_sb[:, :],
                                     op=mybir.AluOpType.subtract).then_inc(s_vch, 1)  # z
                vc += 1
                vector.wait_ge(s_vch, vc)
                vector.tensor_tensor(out=s2_sb[:, :], in0=s1_sb[:, :], in1=s1_sb[:, :],
                                     op=mybir.AluOpType.mult).then_inc(s_vch, 1)  # z^2
                vc += 1
                vector.wait_ge(s_vch, vc)
                vector.tensor_reduce(
                    out=ss_sb[:, :],
                    in_=s2_sb.ap().rearrange("p (g c) -> p g c", c=D),
                    axis=mybir.AxisListType.X, op=mybir.AluOpType.add,
                ).then_inc(s_d2a, 1)
                vector.wait_ge(s_a2d, k * 2 + 1)
                vector.reciprocal(out=rinv_sb[:, :], in_=norm_sb[:, :]).then_inc(s_d2a, 1)
                vector.wait_ge(s_a2d, k * 2 + 2)
                for g in range(groups):
                    gsl = slice(g * D, (g + 1) * D)
                    op = vector.tensor_scalar(
                        out=s2_sb[:, gsl], in0=s1_sb[:, gsl],
                        scalar1=scale_sb[:, g:g + 1], scalar2=None,
                        op0=mybir.AluOpType.mult)
                op.then_inc(s_vch, 1)
                vc += 1
                vector.wait_ge(s_vch, vc)
                vector.tensor_tensor(out=x_sb[:, :], in0=s2_sb[:, :], in1=feat_sb[:, :],
                                     op=mybir.AluOpType.add).then_inc(s_d2a, 1)

        @block.scalar
        def _(scalar: bass.BassEngine):
            for k in range(K_ITERS):
                scalar.wait_ge(s_d2a, k * 3 + 1)
                scalar.activation(out=norm_sb[:, :], in_=ss_sb[:, :],
                                  func=mybir.ActivationFunctionType.Sqrt,
                                  bias=1e-30).then_inc(s_a2d, 1)
                scalar.wait_ge(s_d2a, k * 3 + 2)
                scalar.activation(out=scale_sb[:, :], in_=rinv_sb[:, :],
                                  func=mybir.ActivationFunctionType.Relu,
                                  bias=1.0, scale=-float(GL)).then_inc(s_a2d, 1)
                if k < K_ITERS - 1:
                    scalar.wait_ge(s_d2a, k * 3 + 3)
                    if k > 0:
                        scalar.wait_ge(s_wb, 16 * k)
                    scalar.activation(out=xbf_sb[:, :], in_=x_sb[:, :],
                                      func=mybir.ActivationFunctionType.Copy).then_inc(s_a2s, 1)

    nc.compile()
    return nc


# ----------------------------------------------------------------------------
# public entry point
# ----------------------------------------------------------------------------

def _install_ntff_hook_shim():
    """Provide antenv.axon_hooks (missing in this image) so
    run_bass_kernel_spmd(trace=True) can capture an NTFF profile."""
    import sys, types
    try:
        import antenv.axon_hooks  # noqa: F401
        return
    except ImportError:
        pass
    if "antenv.axon_hooks" in sys.modules:
        return
    try:
        from trn_agent_boot.trn_boot import _ntff_profile_via_ctypes
        hook = _ntff_profile_via_ctypes("/opt/axon/libaxon_pjrt.so")
    except Exception:
        hook = None
    m = types.ModuleType("antenv.axon_hooks")
    m.get_axon_ntff_profile_hook = lambda: hook
    m.set_axon_ntff_profile_hook = lambda h: None
    sys.modules["antenv.axon_hooks"] = m


def kernel(feat, edge_weight, src, dst):
    global last_exec_time_ns
    feat = np.asarray(feat, np.float32)
    edge_weight = np.asarray(edge_weight, np.float32)
    src = np.asarray(src, np.int32)
    dst = np.asarray(dst, np.int32)

    per_core, sched = _preprocess(feat, edge_weight, src, dst)
    nc = _build(sched)

    in_maps = [
        {k: v for k, v in pc.items() if k != "inv"}
        for pc in per_core
    ]
    import os
    if os.environ.get("KERNEL_SIM"):
        import concourse.bass_interp as bass_interp
        sim = bass_interp.MultiCoreSim(nc, NCORES)
        for i in range(NCORES):
            for name, arr in in_maps[i].items():
                sim.cores[i].tensor(name)[:] = arr
        sim.simulate()
        outs = [np.asarray(sim.cores[i].mem_tensor("out")) for i in range(NCORES)]
    else:
        trace = os.environ.get("KERNEL_TRACE", "0") != "0"
        res = None
        if trace:
            try:
                _install_ntff_hook_shim()
                res = run_bass_kernel_spmd(nc, in_maps, core_ids=list(range(NCORES)),
                                           trace=True)
                last_exec_time_ns = res.exec_time_ns
            except Exception:
                res = None
        if res is None:
            res = run_bass_kernel_spmd(nc, in_maps, core_ids=list(range(NCORES)))
        outs = [res.results[k]["out"] for k in range(NCORES)]

    shard = sched["shard"]
    out = np.empty((sched["n"], D), np.float32)
    for k in range(NCORES):
        o = outs[k]  # [spad, D] in slot-permuted order
        inv = per_core[k]["inv"]
        valid = inv >= 0
        out[k * shard + inv[valid]] = o[valid]
    return out


# revision 23
# speedup vs baseline: 1.1724x; 1.0623x over previous
"""Distributed Trainium2 kernel for AdaptiveConv GNN message passing.

Algorithm (per reference): K=3 iterations of
    agg = A_norm @ x            (SpMM over 1.6M edges + self loops)
    z   = agg - feat
    x   = feat + relu(1 - gl/||z||_row) * z
with A_norm the symmetrically-normalized weighted adjacency (self loops
folded into the epilogue analytically).

Mapping to 8 NeuronCores:
  - dst-node 1D partition: core k owns N/8 destination rows (slot-permuted
    for load balance); edges partitioned by dst core.
  - per-iteration AllGather of the bf16 x-table (256B rows).
  - random src-row gather via SWDGE dma_gather (1024 rows / instruction,
    int16 indices => <=32767-row source chunks), edge-on-partition layout.
  - segment-sum via TensorE matmuls with host-precomputed weighted one-hot
    lhsT [128 edges, 128 dst slots] (bf16), PSUM accumulated per dst group,
    SBUF-accumulated across the chunk passes.
  - prox epilogue (row L2 shrinkage) batched on DVE/ACT in f32.
"""
import numpy as np
import ml_dtypes

from concourse import bass, mybir
import concourse.bacc as bacc
from concourse.bass_utils import run_bass_kernel_spmd
from concourse.library_config import mlp

NCORES = 8
D = 50
K_ITERS = 3
LAM = 0.1
GL = (1.0 / (2.0 * (1.0 - LAM))) * LAM  # gamma * lam
EW = 128          # bf16 table row width -> 256B rows
GB_TILES = 8      # tiles per gather slab (1024 index SWDGE limit)
NBUF = 8          # slab buffer rotation depth
NBANKS = 4        # PSUM bank rotation

BF16 = mybir.dt.bfloat16
F32 = mybir.dt.float32
I16 = mybir.dt.int16

last_exec_time_ns = None


# ----------------------------------------------------------------------------
# host-side preprocessing
# ----------------------------------------------------------------------------

def _pack_slots(degc, groups, caps):
    """Assign local dst ids to (group, slot) packing per-chunk in-degree
    vectors degc [n, nchunk] under per-(group, chunk) caps [groups, nchunk].
    Greedy by total degree; score = cap overflow, then max fill fraction.
    Returns pos[local_id] = group*128 + slot."""
    order = np.argsort(-degc.sum(1), kind="stable")
    loads = np.zeros_like(caps)
    cnts = np.zeros(groups, np.int64)
    pos = np.empty(len(degc), np.int64)
    for lid in order:
        nl = loads + degc[lid]
        over = np.maximum(0, nl - caps).sum(axis=1)
        frac = (nl / caps).max(axis=1)
        score = over * 1e6 + frac + (cnts >= 128) * 1e9
        g = int(np.argmin(score))
        pos[lid] = g * 128 + cnts[g]
        loads[g] += degc[lid]
        cnts[g] += 1
    return pos


def _preprocess(feat, edge_weight, src, dst):
    n, d = feat.shape
    assert d == D and n % NCORES == 0
    shard = n // NCORES
    groups = (shard + 127) // 128
    spad = groups * 128
    trows = NCORES * spad

    # chunk boundaries = shard pairs: chunk membership of a source node is
    # then independent of the slot permutation, enabling per-chunk-balanced
    # slot packing against a shared cap template.
    bounds = [0, 2 * spad, 4 * spad, 6 * spad, trows]
    assert 2 * spad <= 32767
    nchunk = len(bounds) - 1
    bounds_arr = np.asarray(bounds)

    # normalization (degrees include self loops with weight 1)
    ew = edge_weight.astype(np.float64)
    out_deg = np.bincount(src, weights=ew, minlength=n) + 1.0
    in_deg = np.bincount(dst, weights=ew, minlength=n) + 1.0
    iso = out_deg ** -0.5
    isi = in_deg ** -0.5
    w = (ew * iso[src] * isi[dst]).astype(np.float32)
    wself = (iso * isi).astype(np.float32)

    # slot permutation per core: pack per-chunk in-degree vectors under a
    # shared cap template so every (chunk, group) needs the same tile count
    # on every core (the SPMD schedule takes the max over cores).
    dcore = dst // shard
    dloc = dst - dcore * shard
    src_chunk = np.searchsorted(bounds_arr, (src // shard) * spad, side="right") - 1
    base_c = np.zeros(nchunk, np.int64)
    extra_c = np.zeros(nchunk, np.int64)
    cnt_kc = np.zeros((NCORES, nchunk), np.int64)
    np.add.at(cnt_kc, (dcore, src_chunk), 1)
    for c in range(nchunk):
        tc = int(-(-(cnt_kc[:, c].max() * 1.03) // 128))
        base_c[c] = max(1, tc // groups)
        extra_c[c] = tc - base_c[c] * groups
    nbig = int(max(0, extra_c.max()))
    caps = np.tile(base_c, (groups, 1)) * 128
    if nbig:
        caps[groups - nbig:, :] += 128
    pos_all = np.empty(n, np.int64)
    for k in range(NCORES):
        m = dcore == k
        degc = np.zeros((shard, nchunk), np.int64)
        np.add.at(degc, (dloc[m], src_chunk[m]), 1)
        pos_all[k * shard:(k + 1) * shard] = _pack_slots(degc, groups, caps)
    row_all = (np.arange(n) // shard) * spad + pos_all  # node -> table row

    srow = row_all[src]
    chunk_of = np.searchsorted(bounds_arr, srow, side="right") - 1
    gid = pos_all[dst] // 128
    slot = pos_all[dst] % 128

    # static tile schedule: Tmax[c][g] = max over cores (>= 1)
    counts = np.zeros((NCORES, nchunk, groups), np.int64)
    np.add.at(counts, (dcore, chunk_of, gid), 1)
    tmax = np.maximum(1, -(-counts.max(axis=0) // 128))  # [nchunk, groups]

    tile_base = np.zeros((nchunk, groups), np.int64)
    seq = []          # (c, g) in schedule order
    tiles = []        # per tile: (c, g, j, seq_idx, start, stop)
    t = 0
    for c in range(nchunk):
        for g in range(groups):
            tile_base[c, g] = t
            tm = int(tmax[c, g])
            si = len(seq)
            for j in range(tm):
                tiles.append((c, g, j, si, j == 0, j == tm - 1))
            seq.append((c, g))
            t += tm
    t_total = t

    # slabs: runs of <= GB_TILES tiles, not crossing chunk boundaries
    slabs = []
    for c in range(nchunk):
        c0 = int(tile_base[c, 0])
        c1 = int(tile_base[c + 1, 0]) if c + 1 < nchunk else t_total
        tt = c0
        while tt < c1:
            nt = min(GB_TILES, c1 - tt)
            slabs.append((tt, nt, c))
            tt += nt
    # groups fully completed once a slab's matmuls are all consumed:
    # 1 + seq index of the group containing the slab's last tile
    slab_complete = [tiles[t0 + nt - 1][3] + 1 for (t0, nt, _) in slabs]

    # per-core data
    per_core = []
    for k in range(NCORES):
        m = dcore == k
        e_srow = srow[m]
        e_chunk = chunk_of[m]
        e_g = gid[m]
        e_slot = slot[m]
        e_w = w[m]

        ordk = np.lexsort((e_slot, e_g, e_chunk))
        e_srow, e_chunk, e_g, e_slot, e_w = (
            a[ordk] for a in (e_srow, e_chunk, e_g, e_slot, e_w))
        cnt_k = np.zeros((nchunk, groups), np.int64)
        np.add.at(cnt_k, (e_chunk, e_g), 1)
        assert np.all(cnt_k <= tmax * 128)
        starts = np.zeros(nchunk * groups, np.int64)
        starts[1:] = np.cumsum(cnt_k.ravel())[:-1]
        flat_cg = e_chunk * groups + e_g
        q = np.arange(len(e_w)) - starts[flat_cg]
        tile_idx = tile_base[e_chunk, e_g] + q // 128
        p_idx = q % 128

        idx16 = np.zeros((16, t_total * 8), np.int16)
        idx16[p_idx % 16, tile_idx * 8 + p_idx // 16] = (
            e_srow - bounds_arr[e_chunk]).astype(np.int16)
        # trailing-pad trim: mark trailing pad slots of each slab -1 and
        # record the per-slab valid count (fed to num_idxs_reg at runtime)
        has_edge = np.zeros(t_total * 128, bool)
        has_edge[tile_idx * 128 + p_idx] = True
        gcnt = np.zeros(len(slabs), np.uint32)
        for si_, (t0_, nt_, _c) in enumerate(slabs):
            ne = nt_ * 128
            occ = has_edge[t0_ * 128:t0_ * 128 + ne]
            nz = np.nonzero(occ)[0]
            valid = int(nz[-1]) + 1 if len(nz) else 0
            gcnt[si_] = valid
        idx_arr = np.tile(idx16, (8, 1))

        oneh = np.zeros((128, t_total, 128), np.float32)
        oneh[p_idx, tile_idx, e_slot] = e_w
        oneh_arr = np.ascontiguousarray(
            oneh.reshape(128, t_total * 128)).astype(ml_dtypes.bfloat16)

        inv = np.full(spad, -1, np.int64)
        inv[pos_all[k * shard:(k + 1) * shard]] = np.arange(shard)
        valid = inv >= 0
        feat_pad = np.zeros((spad, D), np.float32)
        feat_pad[valid] = feat[k * shard + inv[valid]]
        wself_pad = np.zeros(spad, np.float32)
        wself_pad[valid] = wself[k * shard + inv[valid]]

        feat_ep = np.ascontiguousarray(
            feat_pad.reshape(groups, 128, D).transpose(1, 0, 2).reshape(128, groups * D))
        wself_bb = np.ascontiguousarray(np.repeat(
            wself_pad.reshape(groups, 128).T[:, :, None], D, axis=2
        ).reshape(128, groups * D))

        xinit = np.zeros((spad, EW), ml_dtypes.bfloat16)
        xinit[:, :D] = feat_pad.astype(ml_dtypes.bfloat16)

        per_core.append(dict(
            idx=idx_arr, oneh=oneh_arr, feat_ep=feat_ep,
            wselfb=wself_bb.astype(np.float32), xinit=xinit, inv=inv,
            gcnt=gcnt.reshape(1, -1),
        ))

    sched = dict(
        n=n, shard=shard, groups=groups, spad=spad, trows=trows,
        bounds=bounds, nchunk=nchunk, seq=seq, tiles=tiles,
        t_total=t_total, slabs=slabs, slab_complete=slab_complete,
    )
    return per_core, sched


# ----------------------------------------------------------------------------
# device kernel builder
# ----------------------------------------------------------------------------

def _build(sched):
    groups = sched["groups"]
    spad = sched["spad"]
    trows = sched["trows"]
    bounds = sched["bounds"]
    seq = sched["seq"]
    tiles = sched["tiles"]
    slabs = sched["slabs"]
    slab_complete = sched["slab_complete"]
    t_total = sched["t_total"]
    NS = len(slabs)
    NG = len(seq)
    GD = groups * D

    nc = bacc.Bacc("TRN2", num_devices=NCORES)

    tiny = nc.alloc_sbuf_tensor("const-tiny", [128, 1], F32)
    nc.gpsimd.memset(tiny.ap(), 1e-30)
    nc.const_aps.aps[(F32, 1e-30)] = tiny.ap()
    nc.all_engine_barrier()

    idx_ext = nc.declare_dram_parameter("idx", [128, t_total * 8], I16, isOutput=False)
    oneh_ext = nc.declare_dram_parameter("oneh", [128, t_total * 128], BF16, isOutput=False)
    feat_ext = nc.declare_dram_parameter("feat_ep", [128, GD], F32, isOutput=False)
    wself_ext = nc.declare_dram_parameter("wselfb", [128, GD], F32, isOutput=False)
    xinit_ext = nc.declare_dram_parameter("xinit", [spad, EW], BF16, isOutput=False)
    gcnt_ext = nc.declare_dram_parameter("gcnt", [1, NS], mybir.dt.uint32, isOutput=False)
    out_ext = nc.declare_dram_parameter("out", [spad, D], F32, isOutput=True)

    x_bounce = nc.dram_tensor("x_bounce", [spad, EW], BF16)
    x_table = nc.dram_tensor("x_table", [trows, EW], BF16, addr_space="Shared")

    from contextlib import ExitStack
    with ExitStack() as ctx:
        block = ctx.enter_context(nc.Block())
        sem = lambda nm: ctx.enter_context(nc.semaphore(nm))
        sbuf = lambda nm, shp, dt: ctx.enter_context(nc.sbuf_tensor(nm, shp, dt))
        s_init, s_pe, s_dve = sem("s_init"), sem("s_pe"), sem("s_dve")
        s_idx = [sem(f"s_idx{i}") for i in range(NBUF)]
        s_oh = [sem(f"s_oh{i}") for i in range(NBUF)]
        s_gath = [sem(f"s_gath{i}") for i in range(NBUF)]
        s_d2a, s_a2d, s_a2s, s_wb, s_cc, s_vch = (
            sem("s_d2a"), sem("s_a2d"), sem("s_a2s"), sem("s_wb"), sem("s_cc"),
            sem("s_vch"))
        gath_buf = sbuf("gath_buf", [128, NBUF * GB_TILES, EW], BF16)
        oneh_buf = sbuf("oneh_buf", [128, NBUF * GB_TILES * 128], BF16)
        idx_buf = sbuf("idx_buf", [128, NBUF * GB_TILES * 8], I16)
        feat_sb = sbuf("feat_sb", [128, GD], F32)
        wself_sb = sbuf("wself_sb", [128, GD], F32)
        x_sb = sbuf("x_sb", [128, GD], F32)
        agg_sb = sbuf("agg_sb", [128, GD], F32)
        s1_sb = sbuf("s1_sb", [128, GD], F32)
        s2_sb = sbuf("s2_sb", [128, GD], F32)
        xbf_sb = sbuf("xbf_sb", [128, GD], BF16)
        ss_sb = sbuf("ss_sb", [128, groups], F32)
        norm_sb = sbuf("norm_sb", [128, groups], F32)
        rinv_sb = sbuf("rinv_sb", [128, groups], F32)
        scale_sb = sbuf("scale_sb", [128, groups], F32)
        gcnt_sb = sbuf("gcnt_sb", [1, NS], mybir.dt.uint32)
        psum = [ctx.enter_context(nc.psum_tensor(f"psum{i}", [128, 512], F32))
                for i in range(NBANKS)]


        @block.sync
        def _(sync: bass.BassEngine):
            sync.dma_start(out=feat_sb[:, :], in_=feat_ext[:, :]).then_inc(s_init, 16)
            sync.dma_start(out=wself_sb[:, :], in_=wself_ext[:, :]).then_inc(s_init, 16)
            sync.dma_start(out=x_bounce[:, :], in_=xinit_ext[:, :]).then_inc(s_init, 16)
            sync.dma_start(out=gcnt_sb[:, :], in_=gcnt_ext[:, :]).then_inc(s_init, 16)
            for k in range(K_ITERS):
                for s, (t0, nt, c) in enumerate(slabs):
                    gs = k * NS + s
                    b = gs % NBUF
                    if gs >= NBUF:
                        # idx buffer consumed by the gather instruction itself
                        sync.wait_ge(s_gath[b], 16 * (gs // NBUF))
                    sync.dma_start(
                        out=idx_buf[:, b * (GB_TILES * 8):b * (GB_TILES * 8) + nt * 8],
                        in_=idx_ext[:, t0 * 8:(t0 + nt) * 8],
                    ).then_inc(s_idx[b], 16)
                    if gs >= NBUF:
                        # oneh buffer consumed by tensor engine matmuls
                        prev = gs - NBUF
                        sync.wait_ge(s_pe, (prev // NS) * NG + slab_complete[prev % NS])
                    sync.dma_start(
                        out=oneh_buf[:, (b * GB_TILES) * 128:(b * GB_TILES + nt) * 128],
                        in_=oneh_ext[:, t0 * 128:(t0 + nt) * 128],
                    ).then_inc(s_oh[b], 16)
                if k < K_ITERS - 1:
                    sync.wait_ge(s_a2s, k + 1)
                    sync.dma_start(
                        out=x_bounce.ap()[:, :D].rearrange("(g p) c -> p g c", p=128),
                        in_=xbf_sb.ap().rearrange("p (g c) -> p g c", c=D),
                    ).then_inc(s_wb, 16)
            sync.wait_ge(s_d2a, K_ITERS * 3)
            sync.dma_start(
                out=out_ext.ap().rearrange("(g p) c -> p g c", p=128),
                in_=x_sb.ap().rearrange("p (g c) -> p g c", c=D),
            ).then_inc(s_wb, 16)
            sync.wait_ge(s_wb, 16 * K_ITERS)

        @block.gpsimd
        def _(gpsimd: bass.BassGpSimd):
            gpsimd.load_library(mlp)
            gpsimd.wait_ge(s_init, 65)
            nreg = gpsimd.alloc_register("gcnt_reg")
            for k in range(K_ITERS):
                if k > 0:
                    gpsimd.wait_ge(s_wb, 16 * k)
                gpsimd.collective_compute(
                    "AllGather",
                    mybir.AluOpType.bypass,
                    replica_groups=[list(range(NCORES))],
                    ins=[x_bounce.ap().opt()],
                    outs=[x_table.ap().opt()],
                ).then_inc(s_cc)
                gpsimd.wait_ge(s_cc, k + 1)
                for s, (t0, nt, c) in enumerate(slabs):
                    gs = k * NS + s
                    b = gs % NBUF
                    gpsimd.wait_ge(s_idx[b], 16 * (gs // NBUF + 1))
                    if gs >= NBUF:
                        prev = gs - NBUF
                        gpsimd.wait_ge(s_pe, (prev // NS) * NG + slab_complete[prev % NS])
                    gpsimd.dma_gather(
                        out_ap=gath_buf[:, b * GB_TILES:b * GB_TILES + nt, :],
                        in_ap=x_table[bounds[c]:bounds[c + 1], :],
                        idxs_ap=idx_buf[:, b * (GB_TILES * 8):b * (GB_TILES * 8) + nt * 8],
                        num_idxs=nt * 128,
                        num_idxs_reg=nt * 128,
                        elem_size=EW,
                    ).then_inc(s_gath[b], 16)

        @block.tensor
        def _(tensor: bass.BassEngine):
            for k in range(K_ITERS):
                for s, (t0, nt, c) in enumerate(slabs):
                    gs = k * NS + s
                    b = gs % NBUF
                    tensor.wait_ge(s_gath[b], 16 * (gs // NBUF + 1))
                    tensor.wait_ge(s_oh[b], 16 * (gs // NBUF + 1))
                    for j in range(nt):
                        t = t0 + j
                        _, g, _, si, is_start, is_stop = tiles[t]
                        gsi = k * NG + si
                        bank = si % NBANKS
                        if is_start and gsi >= NBANKS:
                            tensor.wait_ge(s_dve, gsi - NBANKS + 1)
                        mm = tensor.matmul(
                            out=psum[bank][:, 0:D],
                            lhsT=oneh_buf[:, (b * GB_TILES + j) * 128:(b * GB_TILES + j + 1) * 128],
                            rhs=gath_buf[:, b * GB_TILES + j, 0:D],
                            start=is_start, stop=is_stop,
                            tile_position=(0, 0),
                        )
                        if is_stop:
                            mm.then_inc(s_pe, 1)

        @block.vector
        def _(vector: bass.BassEngine):
            vc = 0
            vector.memset(gath_buf[:, :, :], 0.0).then_inc(s_init, 1)
            vector.wait_ge(s_init, 65)
            for k in range(K_ITERS):
                for i, (c, g) in enumerate(seq):
                    bank = i % NBANKS
                    vector.wait_ge(s_pe, k * NG + i + 1)
                    if c > 0:
                        # prior drain of same group must have landed
                        vector.wait_ge(s_dve, k * NG + i - groups + 1)
                    gsl = slice(g * D, (g + 1) * D)
                    if c == 0:
                        op = vector.tensor_copy(out=agg_sb[:, gsl], in_=psum[bank][:, 0:D])
                    else:
                        op = vector.tensor_tensor(
                            out=agg_sb[:, gsl], in0=agg_sb[:, gsl],
                            in1=psum[bank][:, 0:D], op=mybir.AluOpType.add)
                    op.then_inc(s_dve, 1)
                # epilogue (batched, f32); wait for own drain writes to land
                vector.wait_ge(s_dve, (k + 1) * NG)
                xsrc = feat_sb if k == 0 else x_sb
                vector.tensor_tensor(out=s1_sb[:, :], in0=xsrc[:, :], in1=wself_sb[:, :],
                                     op=mybir.AluOpType.mult).then_inc(s_vch, 1)
                vc += 1
                vector.wait_ge(s_vch, vc)
                vector.tensor_tensor(out=s2_sb[:, :], in0=agg_sb[:, :], in1=s1_sb[:, :],
                                     op=mybir.AluOpType.add).then_inc(s_vch, 1)
                vc += 1
                vector.wait_ge(s_vch, vc)
                vector.tensor_tensor(out=s1_sb[:, :], in0=s2_sb[:, :], in1=feat_sb[:, :],
                                     op=mybir.AluOpType.subtract).then_inc(s_vch, 1)  # z
                vc += 1
                vector.wait_ge(s_vch, vc)
                vector.tensor_tensor(out=s2_sb[:, :], in0=s1_sb[:, :], in1=s1_sb[:, :],
                                     op=mybir.AluOpType.mult).then_inc(s_vch, 1)  # z^2
                vc += 1
                vector.wait_ge(s_vch, vc)
                vector.tensor_reduce(
                    out=ss_sb[:, :],
                    in_=s2_sb.ap().rearrange("p (g c) -> p g c", c=D),
                    axis=mybir.AxisListType.X, op=mybir.AluOpType.add,
                ).then_inc(s_d2a, 1)
                vector.wait_ge(s_a2d, k * 2 + 1)
                vector.reciprocal(out=rinv_sb[:, :], in_=norm_sb[:, :]).then_inc(s_d2a, 1)
                vector.wait_ge(s_a2d, k * 2 + 2)
                for g in range(groups):
                    gsl = slice(g * D, (g + 1) * D)
                    op = vector.tensor_scalar(
                        out=s2_sb[:, gsl], in0=s1_sb[:, gsl],
                        scalar1=scale_sb[:, g:g + 1], scalar2=None,
                        op0=mybir.AluOpType.mult)
                op.then_inc(s_vch, 1)
                vc += 1
                vector.wait_ge(s_vch, vc)
                vector.tensor_tensor(out=x_sb[:, :], in0=s2_sb[:, :], in1=feat_sb[:, :],
                                     op=mybir.AluOpType.add).then_inc(s_d2a, 1)

        @block.scalar
        def _(scalar: bass.BassEngine):
            for k in range(K_ITERS):
                scalar.wait_ge(s_d2a, k * 3 + 1)
                scalar.activation(out=norm_sb[:, :], in_=ss_sb[:, :],
                                  func=mybir.ActivationFunctionType.Sqrt,
                                  bias=1e-30).then_inc(s_a2d, 1)
                scalar.wait_ge(s_d2a, k * 3 + 2)
                scalar.activation(out=scale_sb[:, :], in_=rinv_sb[:, :],
                                  func=mybir.ActivationFunctionType.Relu,
                                  bias=1.0, scale=-float(GL)).then_inc(s_a2d, 1)
                if k < K_ITERS - 1:
                    scalar.wait_ge(s_d2a, k * 3 + 3)
                    if k > 0:
                        scalar.wait_ge(s_wb, 16 * k)
                    scalar.activation(out=xbf_sb[:, :], in_=x_sb[:, :],
                                      func=mybir.ActivationFunctionType.Copy).then_inc(s_a2s, 1)

    nc.compile()
    return nc


# ----------------------------------------------------------------------------
# public entry point
# ----------------------------------------------------------------------------

def _install_ntff_hook_shim():
    """Provide antenv.axon_hooks (missing in this image) so
    run_bass_kernel_spmd(trace=True) can capture an NTFF profile."""
    import sys, types
    try:
        import antenv.axon_hooks  # noqa: F401
        return
    except ImportError:
        pass
    if "antenv.axon_hooks" in sys.modules:
        return
    try:
        from trn_agent_boot.trn_boot import _ntff_profile_via_ctypes
        hook = _ntff_profile_via_ctypes("/opt/axon/libaxon_pjrt.so")
    except Exception:
        hook = None
    m = types.ModuleType("antenv.axon_hooks")
    m.get_axon_ntff_profile_hook = lambda: hook
    m.set_axon_ntff_profile_hook = lambda h: None
    sys.modules["antenv.axon_hooks"] = m


def kernel(feat, edge_weight, src, dst):
    global last_exec_time_ns
    feat = np.asarray(feat, np.float32)
    edge_weight = np.asarray(edge_weight, np.float32)
    src = np.asarray(src, np.int32)
    dst = np.asarray(dst, np.int32)

    per_core, sched = _preprocess(feat, edge_weight, src, dst)
    nc = _build(sched)

    in_maps = [
        {k: v for k, v in pc.items() if k != "inv"}
        for pc in per_core
    ]
    import os
    if os.environ.get("KERNEL_SIM"):
        import concourse.bass_interp as bass_interp
        sim = bass_interp.MultiCoreSim(nc, NCORES)
        for i in range(NCORES):
            for name, arr in in_maps[i].items():
                sim.cores[i].tensor(name)[:] = arr
        sim.simulate()
        outs = [np.asarray(sim.cores[i].mem_tensor("out")) for i in range(NCORES)]
    else:
        trace = os.environ.get("KERNEL_TRACE", "0") != "0"
        res = None
        if trace:
            try:
                _install_ntff_hook_shim()
                res = run_bass_kernel_spmd(nc, in_maps, core_ids=list(range(NCORES)),
                                           trace=True)
                last_exec_time_ns = res.exec_time_ns
            except Exception:
                res = None
        if res is None:
            res = run_bass_kernel_spmd(nc, in_maps, core_ids=list(range(NCORES)))
        outs = [res.results[k]["out"] for k in range(NCORES)]

    shard = sched["shard"]
    out = np.empty((sched["n"], D), np.float32)
    for k in range(NCORES):
        o = outs[k]  # [spad, D] in slot-permuted order
        inv = per_core[k]["inv"]
        valid = inv >= 0
        out[k * shard + inv[valid]] = o[valid]
    return out
